# revision 1
# baseline (speedup 1.0000x reference)
"""Trainium2 Bass kernel for the dendritic template-gated FFN.

Math (token n, output feature h; W=16 windows of K=64 input features):
    s[n,h,w] = <x[n, w*64:(w+1)*64], template[h, w*64:(w+1)*64]>
    out[n,h] = sum_w softmax_w(s) * silu(s)
             = (sum_w e^{s_w} silu(s_w)) / (sum_w e^{s_w})

Sharding: data-parallel over tokens, 512 per NeuronCore x 8 cores; the
template (8MB) is replicated and held in SBUF (transposed) per core.

Per-core pipeline (all engines busy concurrently; ~82% ACT occupancy):
  PE   : per-window matmuls s_w = x_w^T.T @ t_w^T (K=64 on partitions,
         float32r operands -> full PE rate with ~2^-13 rounding).
         Inputs are PE-transposed into K-major SBUF tiles in a prologue.
  ACT  : e = exp(s);  th = tanh(s/2)      (one table set: exp_and_others)
  DVE  : m = s * e                         (frees the s PSUM slot early)
  GPSIMD: q = th * m   (tensor_tensor; 6 of 8 groups; rest on DVE)
  PE   : window reduction via identity-matmul PSUM accumulation:
           den += (2I) @ e_w ;  num += I @ m_w ; num += I @ q_w
         Since m(1+th) = 2 e silu:  num/den == out exactly.
  DVE  : r = 1/den (2-ULP approx chain), out = num * r -> DMA to DRAM.

The emission is a flattened software pipeline over (region, group): each
group's reduction matmuls are deferred LAG groups so the strictly-FIFO PE
queue never stalls behind the ACT->DVE->GPSIMD epilogue chain.

Numerics: float32r (rounded fp32, ~tf32) matmuls + fp32 everywhere else;
measured end-to-end max rel err vs the fp32 reference: ~3e-4.
"""

import numpy as np
from contextlib import ExitStack

import concourse.bass as bass
import concourse.bacc as bacc
import concourse.mybir as mybir
import concourse.tile as tile
from concourse.bass_utils import run_bass_kernel_spmd

AF = mybir.ActivationFunctionType
ALU = mybir.AluOpType
DT = mybir.dt

N_TOTAL = 4096
IN_F = 1024
OUT_F = 2048
WIN = 64
NW = 16          # windows
N_CORES = 8
N_SH = N_TOTAL // N_CORES   # 512 tokens per core

# windows per PSUM group (2 banks of s -> 3 slots + den/num = 8 banks)
GROUPS = [(0, 1), (2, 3), (4, 5), (6, 7), (8, 9), (10, 11), (12, 13), (14, 15)]
GW = 2


def build_program(n_tok=N_SH, mm_dt=DT.float32r, red_dt=DT.float32r, use_gpsimd=True):
    """Build the single-core Bass program (same NEFF runs SPMD on all cores)."""
    nc = bacc.Bacc(
        "TRN2",
        target_bir_lowering=False,
        debug=False,
        enable_asserts=False,
        num_devices=N_CORES,
    )
    x_d = nc.dram_tensor("x", [n_tok, IN_F], DT.float32, kind="ExternalInput").ap()
    t_d = nc.dram_tensor(
        "template_flat", [OUT_F, IN_F], DT.float32, kind="ExternalInput"
    ).ap()
    eye_d = nc.dram_tensor("eye12", [128, 256], DT.float32, kind="ExternalInput").ap()
    out_d = nc.dram_tensor("out", [n_tok, OUT_F], DT.float32, kind="ExternalOutput").ap()

    NT = n_tok // 128       # token tiles
    NJ = OUT_F // 512       # h chunks
    KB = IN_F // 128        # 128-wide k blocks (2 windows each)
    HB = OUT_F // 128       # 128-row h blocks of template

    # DVE reads of reduced-precision tiles go through an fp32 view (bit-identical)
    def dvr(ap):
        return ap.bitcast(DT.float32) if ap.dtype == DT.float32r else ap

    with ExitStack() as ctx:
        tc = ctx.enter_context(tile.TileContext(nc))

        const_pool = ctx.enter_context(tc.tile_pool(name="const", bufs=1))
        eye_t = const_pool.tile([128, 256], DT.float32, tag="eye")
        nc.sync.dma_start(eye_t[:], eye_d[:])
        eye1 = eye_t[:, 0:128]     # identity (fp32: for PE transposes)
        # rounded copy of [I | 2I] for the reduction matmuls
        eye_r = const_pool.tile([128, 256], red_dt, tag="eyer")
        nc.vector.tensor_copy(eye_r[:], eye_t[:])
        eyer1 = eye_r[:, 0:128]    # identity
        eyer2 = eye_r[:, 128:256]  # 2 * identity

        persist = ctx.enter_context(tc.tile_pool(name="persist", bufs=1))
        xT = [persist.tile([128, n_tok], mm_dt, tag=f"xT{kb}", name=f"xT{kb}")
              for kb in range(KB)]
        tT = [persist.tile([128, OUT_F], mm_dt, tag=f"tT{kb}", name=f"tT{kb}")
              for kb in range(KB)]

        # ---- prologue: load x and template, PE-transpose to K-major ----
        with tc.tile_pool(name="stage", bufs=1) as stage, \
             tc.tile_pool(name="tpsum", bufs=4, space="PSUM") as tpsum:
            x_re = x_d.rearrange("(i p) k -> p i k", p=128)
            t_re = t_d.rearrange("(h p) k -> p h k", p=128)
            t_nm = [stage.tile([128, IN_F], DT.float32, tag=f"tnm{hb}",
                               name=f"tnm{hb}") for hb in range(HB)]
            x_nm = [stage.tile([128, IN_F], DT.float32, tag=f"xnm{i}",
                               name=f"xnm{i}") for i in range(NT)]
            for hb in range(4):
                nc.sync.dma_start(t_nm[hb][:], t_re[:, hb, :])
            for i in range(NT):
                nc.sync.dma_start(x_nm[i][:], x_re[:, i, :])
            for hb in range(4, HB):
                nc.sync.dma_start(t_nm[hb][:], t_re[:, hb, :])

            def do_x(kb):
                ps = tpsum.tile([128, NT * 128], DT.float32, tag="tp", name="psx")
                for i in range(NT):
                    nc.tensor.transpose(
                        ps[:, i * 128:(i + 1) * 128],
                        x_nm[i][:, kb * 128:(kb + 1) * 128],
                        eye1,
                    )
                nc.vector.tensor_copy(xT[kb][:], ps[:])

            def do_t(hq, kb):
                ps = tpsum.tile([128, 512], DT.float32, tag="tp", name="pst")
                for q in range(4):
                    nc.tensor.transpose(
                        ps[:, q * 128:(q + 1) * 128],
                        t_nm[hq + q][:, kb * 128:(kb + 1) * 128],
                        eye1,
                    )
                nc.vector.tensor_copy(tT[kb][:, hq * 128:(hq + 4) * 128],
                                      ps[:, :512])

            for kb in range(KB):
                do_t(0, kb)
            for kb in range(KB):
                do_x(kb)
            for hq in range(4, HB, 4):
                for kb in range(KB):
                    do_t(hq, kb)

        # ---- main pools ----
        spool = ctx.enter_context(tc.tile_pool(name="spsum", bufs=3, space="PSUM"))
        dnpool = ctx.enter_context(tc.tile_pool(name="dnpsum", bufs=1, space="PSUM"))
        e_pool = ctx.enter_context(tc.tile_pool(name="epool", bufs=4))
        th_pool = ctx.enter_context(tc.tile_pool(name="thpool", bufs=3))
        phi_pool = ctx.enter_context(tc.tile_pool(name="phipool", bufs=4))
        p_pool = ctx.enter_context(tc.tile_pool(name="ppool", bufs=4))
        tail_pool = ctx.enter_context(tc.tile_pool(name="tail", bufs=2))

        # ---- main loop: flattened software pipeline over (region, group).
        # Reduction matmuls for a group are emitted LAG groups later so the
        # PE FIFO never stalls on the ACT->DVE->GPSIMD epilogue chain.
        LAG = 3
        pending = []

        def emit_pending(keep):
            while len(pending) > keep:
                pending.pop(0)()

        for j in range(NJ):
            for i in range(NT):
                dn = dnpool.tile([128, 1024], DT.float32, tag="dn")
                den = dn[:, 0:512]
                num = dn[:, 512:1024]
                for gi, grp in enumerate(GROUPS):
                    gl = len(grp) * 512
                    st = spool.tile([128, GW * 512], DT.float32, tag="s")
                    for widx, w in enumerate(grp):
                        base = (w % 2) * 64
                        lhsT = xT[w // 2][base:base + 64, i * 128:(i + 1) * 128]
                        rhs = tT[w // 2][base:base + 64, j * 512:(j + 1) * 512]
                        nc.tensor.matmul(
                            st[:, widx * 512:(widx + 1) * 512],
                            lhsT, rhs,
                            start=True, stop=True, skip_group_check=True,
                        )
                    emit_pending(LAG)
                    e_t = e_pool.tile([128, GW * 512], red_dt, tag="e")
                    nc.scalar.activation(e_t[:, :gl], st[:, :gl], AF.Exp)
                    # m = e * s  (reads st; releases the psum slot early)
                    m_t = phi_pool.tile([128, GW * 512], red_dt, tag="m")
                    nc.vector.tensor_tensor(m_t[:, :gl], st[:, :gl],
                                            dvr(e_t[:, :gl]), ALU.mult)
                    th_t = th_pool.tile([128, GW * 512], DT.float32, tag="th")
                    nc.scalar.activation(th_t[:, :gl], st[:, :gl], AF.Tanh,
                                         scale=0.5)
                    # q = th * m; num accumulates m + q via two matmuls, so
                    # num = sum_w m(1+th) = sum_w 2 e silu.  q runs mostly on
                    # GPSIMD (tensor_tensor is the only Pool-legal form).
                    q_t = p_pool.tile([128, GW * 512], red_dt, tag="p")
                    qeng = nc.gpsimd if (use_gpsimd and gi % 4 != 3) else nc.vector
                    qeng.tensor_tensor(q_t[:, :gl], th_t[:, :gl],
                                       dvr(m_t[:, :gl]), ALU.mult)

                    def red_task(den=den, num=num, e_t=e_t, m_t=m_t, q_t=q_t,
                                 grp=grp):
                        for widx, w in enumerate(grp):
                            sl = slice(widx * 512, (widx + 1) * 512)
                            nc.tensor.matmul(
                                den, eyer2, e_t[:, sl],
                                start=(w == 0), stop=(w == NW - 1),
                                skip_group_check=True,
                            )
                            nc.tensor.matmul(
                                num, eyer1, m_t[:, sl],
                                start=(w == 0), stop=False,
                                skip_group_check=True,
                            )
                            nc.tensor.matmul(
                                num, eyer1, q_t[:, sl],
                                start=False, stop=(w == NW - 1),
                                skip_group_check=True,
                            )
                    pending.append(red_task)

                def tail_task(j=j, i=i, dn=dn):
                    # copy den/num off PSUM so the dn bank frees quickly
                    dv_t = tail_pool.tile([128, 1024], DT.float32, tag="dv")
                    nc.vector.tensor_copy(dv_t[:], dn[:])
                    r_t = tail_pool.tile([128, 512], DT.float32, tag="r")
                    sc_t = tail_pool.tile([128, 512], DT.float32, tag="sc")
                    nc.vector.reciprocal_approx_accurate(
                        r_t[:], dv_t[:, 0:512], scratch=sc_t[:]
                    )
                    o_t = tail_pool.tile([128, 512], DT.float32, tag="o")
                    oeng = nc.gpsimd if use_gpsimd else nc.vector
                    oeng.tensor_tensor(o_t[:], dv_t[:, 512:1024], r_t[:], ALU.mult)
                    nc.sync.dma_start(
                        out_d[i * 128:(i + 1) * 128, j * 512:(j + 1) * 512], o_t[:]
                    )
                pending.append(tail_task)
        emit_pending(0)

    nc.compile()
    return nc


_EYE = None
_PROG = None


def _eye_input():
    global _EYE
    if _EYE is None:
        e = np.eye(128, dtype=np.float32)
        _EYE = np.concatenate([e, 2.0 * e], axis=1)
    return _EYE


def kernel(x: np.ndarray, template_flat: np.ndarray) -> np.ndarray:
    global _PROG
    x = np.ascontiguousarray(x, dtype=np.float32)
    template_flat = np.ascontiguousarray(template_flat, dtype=np.float32)
    assert x.shape == (N_TOTAL, IN_F) and template_flat.shape == (OUT_F, IN_F)
    if _PROG is None:
        _PROG = build_program()
    eye = _eye_input()
    in_maps = [
        {
            "x": x[c * N_SH:(c + 1) * N_SH],
            "template_flat": template_flat,
            "eye12": eye,
        }
        for c in range(N_CORES)
    ]
    res = run_bass_kernel_spmd(_PROG, in_maps, core_ids=list(range(N_CORES)))
    return np.concatenate([r["out"] for r in res.results], axis=0)



# revision 21
# speedup vs baseline: 1.1722x; 1.1722x over previous
"""Trainium2 Bass kernel for the dendritic template-gated FFN.

Math (token n, output feature h; W=16 windows of K=64 input features):
    s[n,h,w] = <x[n, w*64:(w+1)*64], template[h, w*64:(w+1)*64]>
    out[n,h] = sum_w softmax_w(s) * silu(s)
             = (sum_w e^{s_w}(1+tanh(s_w/2)) s_w) / (2 sum_w e^{s_w})

Sharding: data-parallel over tokens, 512 per NeuronCore x 8 cores; the
template (replicated) is held K-major in SBUF per core.

Inputs are converted to bf16 on the host and loaded K-major with
transposing DMAs (XBAR tile transpose), so there is no PE/DVE
transposition prologue at all; compute starts ~3us in.

Per-core pipeline (ACT-bound):
  PE   : per-window bf16 score matmuls -> fp32 PSUM s tiles of 3
         windows (1536 cols) so ACT instructions are as large as PSUM
         allows (6 instr/function/tile; 8 PSUM banks = 2 s slots +
         den/num accumulator).
  ACT  : e = exp(s); th = tanh(s/2), both bf16 (one table set)
  DVE  : m = s * e (frees the s PSUM slot early)
  Pool : q = th * m  (bf16; split with DVE per schedule)
  PE   : window reduction via identity-matmul PSUM accumulation (bf16):
           den += (2I) @ e_w ; num += I @ m_w ; num += I @ q_w
         Since m(1+th) = 2 e silu:  num/den == out exactly.
  DVE  : r ~= 1/den (single-instruction approx, ~51 ULP, direct from
         PSUM), out = num * r -> DMA to DRAM.

Numerics: bf16 scores and e/th/m/q, fp32 PSUM accumulation; measured
end-to-end max rel err vs the fp32 reference: ~5e-3 (budget 2e-2).
"""

import numpy as np
import ml_dtypes
from contextlib import ExitStack

import concourse.bass as bass
import concourse.bacc as bacc
import concourse.mybir as mybir
import concourse.tile as tile
from concourse.bass_utils import run_bass_kernel_spmd

AF = mybir.ActivationFunctionType
ALU = mybir.AluOpType
DT = mybir.dt

N_TOTAL = 4096
IN_F = 1024
OUT_F = 2048
WIN = 64
NW = 16          # windows
N_CORES = 8
N_SH = N_TOTAL // N_CORES   # 512 tokens per core

# windows per PSUM s-tile group: gw windows = gw PSUM banks per slot;
# slots * gw + 2 (den/num accumulator) must be <= 8 banks.
def make_groups(gw):
    return [tuple(range(a, min(a + gw, NW))) for a in range(0, NW, gw)]


def build_program(n_tok=N_SH, mm_dt=DT.bfloat16, red_dt=DT.bfloat16,
                  gw=2, sbufs=3, lag=8,
                  qmode=(2, (0, 1, 3, 4, 5), (1, 14)), den_pool=False):
    """Build the single-core Bass program (same NEFF runs SPMD on all cores)."""
    nc = bacc.Bacc(
        "TRN2",
        target_bir_lowering=False,
        debug=False,
        enable_asserts=False,
        num_devices=N_CORES,
    )
    x_d = nc.dram_tensor("x", [n_tok, IN_F], mm_dt, kind="ExternalInput").ap()
    t_d = nc.dram_tensor(
        "template_flat", [OUT_F, IN_F], mm_dt, kind="ExternalInput"
    ).ap()
    eye_d = nc.dram_tensor("eye12", [128, 256], DT.float32, kind="ExternalInput").ap()
    out_d = nc.dram_tensor("out", [n_tok, OUT_F], DT.float32, kind="ExternalOutput").ap()

    NT = n_tok // 128       # token tiles (4)
    NJ = OUT_F // 512       # h chunks (4)
    KB = IN_F // 128        # 128-wide k blocks (2 windows each) (8)
    GROUPS = make_groups(gw)
    GW = gw
    LAG = lag

    with ExitStack() as ctx:
        tc = ctx.enter_context(tile.TileContext(nc))

        # K-major persistent operands, filled by transposing DMAs.
        # xT column layout: kb-block kb at cols [kb*n_tok, (kb+1)*n_tok).
        persist = ctx.enter_context(tc.tile_pool(name="persist", bufs=1))
        xT = persist.tile([128, KB * n_tok], mm_dt, tag="xT", name="xT")
        tT = [persist.tile([128, OUT_F], mm_dt, tag=f"tT{kb}", name=f"tT{kb}")
              for kb in range(KB)]

        # interleave so the kb-blocks needed first arrive first
        for kb in range(KB):
            nc.sync.dma_start_transpose(
                xT[:, kb * n_tok:(kb + 1) * n_tok],
                x_d[:, kb * 128:(kb + 1) * 128])
            nc.sync.dma_start_transpose(
                tT[kb][:], t_d[:, kb * 128:(kb + 1) * 128])

        # eye (for reduction identities) only needed by the first
        # LAG-deferred reduction -> load after the critical input DMAs
        const_pool = ctx.enter_context(tc.tile_pool(name="const", bufs=1))
        eye_t = const_pool.tile([128, 256], DT.float32, tag="eye")
        nc.sync.dma_start(eye_t[:], eye_d[:])
        eye_b = const_pool.tile([128, 384], DT.bfloat16, tag="eyeb")
        nc.vector.tensor_copy(eye_b[:, 0:256], eye_t[:])
        eyeb1 = eye_b[:, 0:128]    # identity
        eyeb2 = eye_b[:, 128:256]  # 2 * identity
        eyeb3 = eye_b[:, 256:384]  # -2 * identity (for the sigma-path num)
        nc.vector.tensor_scalar_mul(eyeb3, eyeb2, -1.0)

        # PE clock warm-up: ~2.5us of dummy transposes starting at t=0 so
        # the p-state ramp (full clock needs 3us of continuous busy) is done
        # before the first score matmuls; targets the dn accumulator bank,
        # which the first den matmul (start=True) later overwrites anyway.
        warm_pool = ctx.enter_context(tc.tile_pool(name="warm", bufs=1))
        warm_t = warm_pool.tile([128, 64], DT.float32, tag="warm")
        nc.vector.memset(warm_t[:], 0.0)

        # ---- main pools ----
        spool = ctx.enter_context(tc.tile_pool(name="spsum", bufs=sbufs, space="PSUM"))
        dnpool = ctx.enter_context(tc.tile_pool(name="dnpsum", bufs=1, space="PSUM"))
        nb = lag + 3
        e_pool = ctx.enter_context(tc.tile_pool(name="epool", bufs=nb))
        th_pool = ctx.enter_context(tc.tile_pool(name="thpool", bufs=4))
        m_pool = ctx.enter_context(tc.tile_pool(name="mpool", bufs=nb))
        q_pool = ctx.enter_context(tc.tile_pool(name="qpool", bufs=nb))
        dp_pool = (ctx.enter_context(tc.tile_pool(name="dppool", bufs=nb))
                   if den_pool else None)
        sig_pool = (ctx.enter_context(tc.tile_pool(name="sigpool", bufs=3))
                    if isinstance(qmode, tuple) else None)
        tail_pool = ctx.enter_context(tc.tile_pool(name="tail", bufs=3))

        # ---- main loop: flattened software pipeline over (j, i, group)
        # with a one-group lookahead: scores for step n+1 are emitted before
        # the epilogue of step n, so the strictly-FIFO PE queue always has
        # the next s-tile ready before ACT finishes the current group.
        # Reduction matmuls are further deferred LAG groups (pending).
        pending = []

        def emit_pending(keep):
            while len(pending) > keep:
                pending.pop(0)()

        steps = [(j, i, gi) for j in range(NJ) for i in range(NT)
                 for gi in range(len(GROUPS))]
        dns = {}
        sts = {}
        warm_dn = dnpool.tile([128, 1024], DT.float32, tag="dn", name="warmdn")
        for _ in range(26):
            nc.tensor.transpose(warm_dn[0:64, 0:64], warm_t[0:64, 0:64],
                                warm_t[0:64, 0:64])

        def emit_scores(n):
            j, i, gi = steps[n]
            grp = GROUPS[gi]
            if gi == 0:
                dns[(j, i)] = dnpool.tile([128, 1024], DT.float32, tag="dn", name="dn")
            st = spool.tile([128, GW * 512], DT.float32, tag="s", name="st")
            sts[n] = st
            for widx, w in enumerate(grp):
                base = (w % 2) * 64
                col = (w // 2) * n_tok + i * 128
                lhsT = xT[base:base + 64, col:col + 128]
                rhs = tT[w // 2][base:base + 64, j * 512:(j + 1) * 512]
                nc.tensor.matmul(
                    st[:, widx * 512:(widx + 1) * 512],
                    lhsT, rhs,
                    start=True, stop=True, skip_group_check=True,
                )

        def emit_epilogue(n):
            j, i, gi = steps[n]
            grp = GROUPS[gi]
            gl = len(grp) * 512
            st = sts.pop(n)
            dn = dns[(j, i)]
            den = dn[:, 0:512]
            num = dn[:, 512:1024]
            e_t = e_pool.tile([128, GW * 512], red_dt, tag="e")
            nc.scalar.activation(e_t[:, :gl], st[:, :gl], AF.Exp)
            # m = s * e  (reads st; releases the psum slot early)
            m_t = m_pool.tile([128, GW * 512], red_dt, tag="m")
            nc.vector.tensor_tensor(m_t[:, :gl], st[:, :gl],
                                    e_t[:, :gl], ALU.mult)
            q_t = q_pool.tile([128, GW * 512], red_dt, tag="q")
            ti = j * NT + i
            sig = (isinstance(qmode, tuple) and gi == qmode[0]
                   and (len(qmode) < 3 or qmode[2][0] <= ti <= qmode[2][1]))
            if sig:
                # sigma path (skips the ACT tanh pass): m(1+th) = 2m - 2mr
                # with r ~= 1/(1+e), since th = 1 - 2/(1+e) exactly. b and
                # mr run on Pool, the reciprocal on DVE; num accumulates
                # 2I @ m and (-2I) @ mr.
                b_t = sig_pool.tile([128, GW * 512], DT.float32, tag="b")
                nc.gpsimd.tensor_scalar_add(b_t[:, :gl], e_t[:, :gl], 1.0)
                r_t = sig_pool.tile([128, GW * 512], DT.float32, tag="rr")
                nc.vector.reciprocal_approx_fast(r_t[:, :gl], b_t[:, :gl])
                nc.gpsimd.tensor_tensor(q_t[:, :gl], m_t[:, :gl],
                                        r_t[:, :gl], ALU.mult)
            else:
                th_t = th_pool.tile([128, GW * 512], red_dt, tag="th")
                nc.scalar.activation(th_t[:, :gl], st[:, :gl], AF.Tanh,
                                     scale=0.5)
                # q = th * m; num accumulates m + q, so
                # num = sum_w m(1+th) = sum_w 2 e silu.
                if qmode == "pool4":
                    qeng = nc.gpsimd if gi < 4 else nc.vector
                elif qmode == "dve":
                    qeng = nc.vector
                elif isinstance(qmode, tuple):
                    qeng = nc.gpsimd if gi in qmode[1] else nc.vector
                else:
                    qeng = nc.gpsimd
                qeng.tensor_tensor(q_t[:, :gl], th_t[:, :gl],
                                   m_t[:, :gl], ALU.mult)

            # den partial-sum on Pool: p = e0+e1(+e2), den += 2I @ p
            # (1 matmul per group instead of per window)
            dp_t = None
            if den_pool and len(grp) > 1:
                dp_t = dp_pool.tile([128, 512], red_dt, tag="dp")
                deng = nc.gpsimd if (den_pool is True or gi in den_pool)                     else nc.vector
                deng.tensor_tensor(dp_t[:], e_t[:, 0:512],
                                   e_t[:, 512:1024], ALU.add)
                for widx in range(2, len(grp)):
                    deng.tensor_tensor(
                        dp_t[:], dp_t[:],
                        e_t[:, widx * 512:(widx + 1) * 512], ALU.add)

            def red_task(den=den, num=num, e_t=e_t, m_t=m_t, q_t=q_t,
                         grp=grp, dp_t=dp_t, sig=sig):
                if dp_t is not None:
                    nc.tensor.matmul(
                        den, eyeb2, dp_t[:],
                        start=(grp[0] == 0), stop=(grp[-1] == NW - 1),
                        skip_group_check=True,
                    )
                for widx, w in enumerate(grp):
                    sl = slice(widx * 512, (widx + 1) * 512)
                    if dp_t is None:
                        nc.tensor.matmul(
                            den, eyeb2, e_t[:, sl],
                            start=(w == 0), stop=(w == NW - 1),
                            skip_group_check=True,
                        )
                    nc.tensor.matmul(
                        num, eyeb2 if sig else eyeb1, m_t[:, sl],
                        start=(w == 0), stop=False,
                        skip_group_check=True,
                    )
                    nc.tensor.matmul(
                        num, eyeb3 if sig else eyeb1, q_t[:, sl],
                        start=False, stop=(w == NW - 1),
                        skip_group_check=True,
                    )
            pending.append(red_task)

            if gi == len(GROUPS) - 1:
                def tail_task(j=j, i=i, dn=dn):
                    # r ~= 1/den straight from PSUM (51 ULP is plenty for
                    # the 2e-2 budget); out = num * r, also from PSUM.
                    r_t = tail_pool.tile([128, 512], DT.float32, tag="r")
                    nc.vector.reciprocal_approx_fast(r_t[:], dn[:, 0:512])
                    o_t = tail_pool.tile([128, 512], DT.float32, tag="o")
                    nc.vector.tensor_tensor(o_t[:], dn[:, 512:1024], r_t[:],
                                            ALU.mult)
                    nc.sync.dma_start(
                        out_d[i * 128:(i + 1) * 128,
                              j * 512:(j + 1) * 512], o_t[:]
                    )
                pending.append(tail_task)

        NSTEP = len(steps)
        emit_scores(0)
        for n in range(NSTEP):
            if n + 1 < NSTEP:
                emit_scores(n + 1)
            emit_epilogue(n)
            # drain the deferral progressively through the last tile so the
            # end-of-schedule serial chain is short
            keep = LAG if n < NSTEP - LAG else NSTEP - 1 - n
            emit_pending(keep)
        emit_pending(0)

    nc.compile()
    return nc


_EYE = None
_PROG = None


def _eye_input():
    global _EYE
    if _EYE is None:
        e = np.eye(128, dtype=np.float32)
        _EYE = np.concatenate([e, 2.0 * e], axis=1)
    return _EYE


def kernel(x: np.ndarray, template_flat: np.ndarray) -> np.ndarray:
    global _PROG
    x = np.ascontiguousarray(x, dtype=np.float32)
    template_flat = np.ascontiguousarray(template_flat, dtype=np.float32)
    assert x.shape == (N_TOTAL, IN_F) and template_flat.shape == (OUT_F, IN_F)
    if _PROG is None:
        _PROG = build_program()
    eye = _eye_input()
    x_bf = x.astype(ml_dtypes.bfloat16)
    t_bf = template_flat.astype(ml_dtypes.bfloat16)
    in_maps = [
        {
            "x": x_bf[c * N_SH:(c + 1) * N_SH],
            "template_flat": t_bf,
            "eye12": eye,
        }
        for c in range(N_CORES)
    ]
    res = run_bass_kernel_spmd(_PROG, in_maps, core_ids=list(range(N_CORES)))
    return np.concatenate([r["out"] for r in res.results], axis=0)
